# revision 41
# baseline (speedup 1.0000x reference)
"""CliffordBatchNormMV Trainium2 kernel (bf16 I/O, mv-major layout, tall
per-token norm math).

Math (per grade g, block nb, batch token b):
  sumsq[g,b] = sum_{c in grade g} x[c,b]^2
  n = sqrt(sumsq + EPS)                      # grade norm
  mean/var over b (biased)                   # batch stats per (g, nb)
  inv = 1/sqrt(var + EPS)
  out[c,b] = x[c,b] * s[g(c),b],  s = A + C/n,  A = gs*gamma*inv,
                                  C = gs*(beta - gamma*inv*mean)

Distribution: shard the 64 nb-blocks across 8 cores (8 each) -> batch stats
are fully core-local, no collectives.

I/O in bf16 with the host pre-transposing each nb-group to mv-major
[256, 4096] (and inverting afterwards): halves HBM traffic vs f32 to a
~93 us/core DMA floor (32 MiB at 360 B/ns).

The per-token norm math runs in a TALL layout [128 tok, 4q, 10 grades]
(tokens on partitions) so sqrt/reciprocal cost free-size 40 instead of
512, and all batch reductions become tiny PE matmuls:
  pass 1 per chunk (512 tok = 4 q-slices of 128):
    x2 = x*x bf16 (ACT Square / gpsimd split); 8 small PE matmuls with the
    x2 q-slice as stationary and the 0/1 grade matrix [128,10] as moving
    -> ps_tall [128,4,10] f32 PSUM; ACT Sqrt(+eps) -> gn_tall f32r; PE
    ones-matmul accumulates sum(n) and a gn^T@gn Gram matmul accumulates
    sum(n^2) per grade across the whole group (PSUM accumulation); DVE
    reciprocal -> rn_tall (col 9 memset to 1.0: the expansion ones-row);
    4 PE permutation-transposes -> rnT [10,512] f32r PSUM; ACT Copy ->
    gradewise rn (kept for pass 2).
  stats per group: mean from the ones-matmul bank, E[n^2] from the Gram
  diagonal; inv via Sqrt + reciprocal; A, C [9,1] f32 exact. E_aug
  [10,256] f32r: rows 0..8 = C[g]*G9, row 9 = A[g(c)] (tiny PE matmul +
  ACT copy + row DMA).
  pass 2 per chunk: f32r expansion matmul pair E_aug^T @ rn_aug ->
  s = A + C*rn [128,2,512] f32 PSUM (exact f32 -> no cancellation error);
  DVE multiplies in-place into the resident x tile; DMA out.

Engine steady-state per chunk ~1.30-1.48 us vs the 1.456 us DMA floor;
single activation table set (sqrt/square/copy/identity) -> no reloads.
"""

import os
import numpy as np

MV = 256
NG = 9
NGP = 10                         # padded grade cols (col 9 = ones trick)
EPS = 1e-5
B = 4096
NB = 64
N_CORES = 8
NB_PER_CORE = NB // N_CORES      # 8 nb-groups per core
NCHUNK = 8                       # 512-token chunks per group
TOK = 512                        # tokens per chunk
NQ = 4                           # 128-token q-slices per chunk

_GRADES = np.array([bin(i).count("1") for i in range(MV)])

LAST_RESULTS = None
_CACHE = {}


def _build_program():
    import concourse.bacc as bacc
    import concourse.tile as tile
    from concourse import mybir

    f32 = mybir.dt.float32
    f32r = mybir.dt.float32r
    bf16 = mybir.dt.bfloat16
    AF = mybir.ActivationFunctionType
    Alu = mybir.AluOpType

    Gp = np.zeros((MV, NGP), dtype=np.float32)
    Gp[np.arange(MV), _GRADES] = 1.0          # 10th column stays zero
    G9 = np.ascontiguousarray(Gp[:, :NG].T)   # [9, 256]

    CPD = int(os.environ.get("K_CPD", "4"))       # chunks per DMA tile
    SKIP = set(os.environ.get("K_SKIP", "").split(","))
    GNQ = int(os.environ.get("K_GNQ", "3"))       # gn-stage pipeline depth
    OFS = int(os.environ.get("K_OFS", "16"))      # pass2 lag in chunk-slots (8..16)
    FLUSH = int(os.environ.get("K_FLUSH", "1"))   # flush gn stage at group end
    QA = int(os.environ.get("K_QA", "224"))       # x2 cols on ACT
    QD = int(os.environ.get("K_QD", "0"))         # x2 cols on DVE (tail)
    QA2 = int(os.environ.get("K_QA2", "96"))      # fill-phase (groups 0-1) ACT cols
    QD2 = int(os.environ.get("K_QD2", "256"))     # fill-phase DVE cols

    nc = bacc.Bacc()
    x_in = nc.dram_tensor("x", [NB_PER_CORE, MV, B], bf16, kind="ExternalInput")
    gg_in = nc.dram_tensor("gg", [NG, NB_PER_CORE], f32, kind="ExternalInput")
    gb_in = nc.dram_tensor("gb", [NG, NB_PER_CORE], f32, kind="ExternalInput")
    out_d = nc.dram_tensor("out", [NB_PER_CORE, MV, B], bf16, kind="ExternalOutput")

    G_lo_c = nc.inline_tensor(Gp[:128], name="Glo")
    G_hi_c = nc.inline_tensor(Gp[128:], name="Ghi")
    G9_c = nc.inline_tensor(G9, name="G9")
    I128_c = nc.inline_tensor(np.eye(128, dtype=np.float32), name="I128")
    I10_c = nc.inline_tensor(np.eye(NGP, dtype=np.float32), name="I10")
    ones_c = nc.inline_tensor(np.ones((128, 8), dtype=np.float32), name="ones")

    inv_B = 1.0 / B

    with tile.TileContext(nc) as tc:
        with (
            tc.tile_pool(name="const", bufs=1) as const,
            tc.tile_pool(name="xc", bufs=int(os.environ.get("K_XC", "10"))) as xcp,
            tc.tile_pool(name="x2p", bufs=int(os.environ.get("K_X2", "4"))) as x2p,
            tc.tile_pool(name="gnp", bufs=int(os.environ.get("K_GN", "5"))) as gnp,
            tc.tile_pool(name="grp", bufs=2) as grp,
            tc.tile_pool(name="statp", bufs=2) as statp,
            tc.tile_pool(name="ps_t", bufs=int(os.environ.get("K_PST", "1")), space="PSUM") as ps_t,
            tc.tile_pool(name="ps_r", bufs=int(os.environ.get("K_PSR", "1")), space="PSUM") as ps_r,
            tc.tile_pool(name="ps_g", bufs=1, space="PSUM") as ps_g,
            tc.tile_pool(name="ps_n", bufs=1, space="PSUM") as ps_n,
            tc.tile_pool(name="ps_x", bufs=int(os.environ.get("K_PSX", "2")), space="PSUM") as ps_x,
        ):
            Gmv_lo = const.tile([128, NGP], bf16)
            nc.gpsimd.dma_start(out=Gmv_lo, in_=G_lo_c[:, :])
            Gmv_hi = const.tile([128, NGP], bf16)
            nc.gpsimd.dma_start(out=Gmv_hi, in_=G_hi_c[:, :])
            G9f = const.tile([NG, MV], f32)
            nc.sync.dma_start(out=G9f, in_=G9_c[:, :])
            G9r = const.tile([NG, MV], f32r)
            nc.gpsimd.dma_start(out=G9r, in_=G9_c[:, :])
            I128 = const.tile([128, 128], f32r)
            nc.gpsimd.dma_start(out=I128, in_=I128_c[:, :])
            I10m = const.tile([NGP, NGP], f32)
            nc.sync.dma_start(out=I10m, in_=I10_c[:, :])
            gg = const.tile([NG, NB_PER_CORE], f32)
            nc.sync.dma_start(out=gg, in_=gg_in[:, :])
            gb = const.tile([NG, NB_PER_CORE], f32)
            nc.sync.dma_start(out=gb, in_=gb_in[:, :])
            ones_f = const.tile([128, 8], f32r)
            nc.gpsimd.dma_start(out=ones_f, in_=ones_c[:, :])
            ones41 = const.tile([128, NQ, 1], f32)
            nc.vector.memset(ones41, 1.0)
            eps_col = const.tile([128, 1], f32)
            nc.vector.memset(eps_col, EPS)
            epsg = const.tile([NGP, 1], f32)
            nc.vector.memset(epsg, EPS)

            NGROUPS = int(os.environ.get("K_NGROUPS", str(NB_PER_CORE)))

            # two-group software pipeline: group g's pass 1 runs in loop g,
            # its post-sqrt stage and batch stats finish early in loop g+1,
            # and its pass 2 runs in loop g+2. Every engine stream then has
            # only ready-or-nearly-ready work, and the stats chain (which
            # includes two high-latency tiny DMAs) is fully hidden.
            ctx = {}
            gn_q = []
            out_done = set()

            def gn_stage(state):
                g_, ch, gn = state
                c = ctx[g_]
                first = (ch == 0)
                last = (ch == NCHUNK - 1)
                rn = gnp.tile([128, NQ, NGP], f32r, tag="rn")
                with nc.allow_low_precision("f32r bits are f32"):
                    nc.vector.reciprocal(out=rn, in_=gn)
                # col 9: sumsq is 0 there -> force rn to 1.0 so the
                # expansion's A-row (ones moving row) works
                nc.vector.tensor_scalar(
                    out=rn[:, :, NG:NGP], in0=ones41, scalar1=1.0,
                    scalar2=None, op0=Alu.mult,
                )
                rnT = ps_r.tile([NGP, TOK], f32r, tag="rnT")
                for q in range(NQ):
                    nc.tensor.transpose(
                        rnT[:, q * 128:(q + 1) * 128], rn[:, q, :], I128
                    )
                nc.scalar.activation(
                    out=c["rgw"][:, ch, :], in_=rnT, func=AF.Copy
                )
                for q in range(NQ):
                    nc.tensor.matmul(
                        c["snb"][0:NGP, 0:8], gn[:, q, :], ones_f,
                        start=(first and q == 0), stop=(last and q == NQ - 1),
                    )
                    nc.tensor.matmul(
                        c["gram"], gn[:, q, :], gn[:, q, :],
                        start=(first and q == 0), stop=(last and q == NQ - 1),
                    )
                if last:
                    emit_stats(g_)

            def emit_stats(g_):
                c = ctx[g_]
                snb = c["snb"]
                gram = c["gram"]
                # -mean column: -(sum n)/B, straight off the PE bank
                mn = statp.tile([NGP, 1], f32, tag="mn")
                nc.vector.tensor_scalar(
                    out=mn, in0=snb[0:NGP, 0:1], scalar1=-inv_B, scalar2=None,
                    op0=Alu.mult,
                )
                # E[n^2]*B per grade = Gram diagonal
                gd = statp.tile([NGP, NGP], f32, tag="gd")
                nc.vector.tensor_mul(gd, gram, I10m)
                sv = statp.tile([NGP, 1], f32, tag="sv")
                nc.vector.tensor_reduce(
                    out=sv, in_=gd, axis=mybir.AxisListType.X, op=Alu.add
                )
                m2 = statp.tile([NGP, 1], f32, tag="m2")
                nc.gpsimd.tensor_mul(m2, mn, mn)
                # var + EPS = sv/B - mean^2 + EPS  (sv already includes +EPS)
                var = statp.tile([NGP, 1], f32, tag="var")
                nc.vector.scalar_tensor_tensor(
                    out=var, in0=sv, scalar=inv_B, in1=m2,
                    op0=Alu.mult, op1=Alu.subtract,
                )
                sd = statp.tile([NGP, 1], f32, tag="sd")
                nc.scalar.activation(
                    out=sd, in_=var, func=AF.Sqrt, bias=epsg[:, 0:1]
                )
                inv = statp.tile([NGP, 1], f32, tag="inv")
                nc.vector.reciprocal(out=inv, in_=sd)
                Ar = statp.tile([NG, 1], f32r, tag="Ar")
                nc.gpsimd.tensor_mul(Ar, gg[:, g_:g_ + 1], inv[0:NG, :])
                Af = statp.tile([NG, 1], f32, tag="Af")
                nc.gpsimd.tensor_mul(Af, gg[:, g_:g_ + 1], inv[0:NG, :])
                C = statp.tile([NG, 1], f32, tag="C")
                nc.vector.scalar_tensor_tensor(
                    out=C, in0=Af, scalar=mn[0:NG, :], in1=gb[:, g_:g_ + 1],
                    op0=Alu.mult, op1=Alu.add,
                )
                # E_aug rows 0..8 = C[g]*G9 ; row 9 = A[g(c)]
                Eaug = statp.tile([NGP, MV], f32r, tag="Eaug")
                nc.vector.tensor_scalar(
                    out=Eaug[0:NG, :], in0=G9f, scalar1=C, scalar2=None,
                    op0=Alu.mult,
                )
                nc.tensor.matmul(snb[0:1, MV:2 * MV], Ar, G9r,
                                 start=True, stop=True)
                aex = statp.tile([1, MV], f32r, tag="aex")
                nc.scalar.activation(
                    out=aex, in_=snb[0:1, MV:2 * MV], func=AF.Copy
                )
                nc.sync.dma_start(out=Eaug[NG:NGP, :], in_=aex)
                c["Eaug"] = Eaug

            def emit_out(gi, h):
                if (gi, h) in out_done:
                    return
                out_done.add((gi, h))
                c = ctx[gi]
                nt = NCHUNK // CPD
                for i in range(nt):
                    sl = slice(i * CPD * TOK, (i + 1) * CPD * TOK)
                    nc.sync.dma_start(out=c["ov"][h, :, sl],
                                      in_=c["xts"][i][:, h, :])

            def pass2_chunk(c, ch):
                co = (ch % CPD) * TOK
                xc = c["xts"][ch // CPD][:, :, co:co + TOK]
                sx = ps_x.tile([128, 2, TOK], f32, tag="sx")
                Eaug_p = c["Eaug"]
                nc.tensor.matmul(sx[:, 0, :], Eaug_p[:, 0:128],
                                 c["rgw"][:, ch, :], start=True, stop=True)
                nc.tensor.matmul(sx[:, 1, :], Eaug_p[:, 128:256],
                                 c["rgw"][:, ch, :], start=True, stop=True)
                if "mult" not in SKIP:
                    nc.vector.tensor_mul(xc, xc, sx)

            for g in range(NGROUPS + (OFS + NCHUNK - 1) // NCHUNK):
                if g < NGROUPS:
                    xv = x_in[g].rearrange("(h p) t -> h p t", h=2)
                    ctx[g] = {
                        "ov": out_d[g].rearrange("(h p) t -> h p t", h=2),
                        "rgw": grp.tile([NGP, NCHUNK, TOK], f32r, tag="rgw", name="rgw"),
                        "gram": ps_g.tile([NGP, NGP], f32, tag="gram", name="gram"),
                        "snb": ps_n.tile([NGP, TOK], f32, tag="snb", name="snb"),
                        "xts": [],
                    }

                for ch in range(NCHUNK):
                    if g < NGROUPS:
                        if ch % CPD == 0:
                            xt = xcp.tile([128, 2, CPD * TOK], bf16, tag="xc")
                            ctx[g]["xts"].append(xt)
                            sl = slice(ch * TOK, (ch + CPD) * TOK)
                            nc.sync.dma_start(out=xt[:, 0, :], in_=xv[0, :, sl])
                            nc.sync.dma_start(out=xt[:, 1, :], in_=xv[1, :, sl])
                        co = (ch % CPD) * TOK
                        xc = ctx[g]["xts"][ch // CPD][:, :, co:co + TOK]

                        x2 = x2p.tile([128, 2, TOK], bf16, tag="x2")
                        qa, qd = (QA2, QD2) if g < 2 else (QA, QD)
                        nc.scalar.activation(
                            out=x2[:, :, 0:qa], in_=xc[:, :, 0:qa],
                            func=AF.Square
                        )
                        nc.gpsimd.tensor_mul(
                            x2[:, :, qa:TOK - qd], xc[:, :, qa:TOK - qd],
                            xc[:, :, qa:TOK - qd],
                        )
                        if qd:
                            nc.vector.tensor_mul(
                                x2[:, :, TOK - qd:TOK], xc[:, :, TOK - qd:TOK],
                                xc[:, :, TOK - qd:TOK],
                            )

                        pst = ps_t.tile([128, NQ, NGP], f32, tag="pst")
                        for q in range(NQ):
                            qs = slice(q * 128, (q + 1) * 128)
                            nc.tensor.matmul(pst[:, q, :], x2[:, 0, qs],
                                             Gmv_lo, start=True, stop=False)
                            nc.tensor.matmul(pst[:, q, :], x2[:, 1, qs],
                                             Gmv_hi, start=False, stop=True)

                        gn = gnp.tile([128, NQ, NGP], f32r, tag="gn")
                        nc.scalar.activation(
                            out=gn, in_=pst, func=AF.Sqrt, bias=eps_col[:, 0:1]
                        )
                        gn_q.append((g, ch, gn))
                        if len(gn_q) > GNQ:
                            gn_stage(gn_q.pop(0))
                        if FLUSH and ch == NCHUNK - 1:
                            while gn_q:
                                gn_stage(gn_q.pop(0))
                    elif gn_q:
                        gn_stage(gn_q.pop(0))

                    # pass2 of group g2 runs OFS chunk-slots behind its
                    # pass 1 (OFS=16 -> full two-group lag; OFS=8..15 ->
                    # fractional, shorter fill/drain)
                    slot = g * NCHUNK + ch - OFS
                    g2, c2 = divmod(slot, NCHUNK)
                    if 0 <= g2 < NGROUPS:
                        pass2_chunk(ctx[g2], c2)
                    # out-DMAs: half h of group gp once its last mult is
                    # ~2 slots old (slot - OFS references pass2 progress)
                    if 0 <= g2 < NGROUPS + 1 and c2 in (2, 4):
                        gp = g2 - 1
                        if 0 <= gp < NGROUPS and gp in ctx:
                            emit_out(gp, 0 if c2 == 2 else 1)

                if g - 4 >= 0:
                    ctx.pop(g - 4, None)

            for gi in range(max(0, NGROUPS - 3), NGROUPS):
                if gi in ctx:
                    emit_out(gi, 0)
                    emit_out(gi, 1)

    nc.compile()
    return nc


def kernel(x, gamma, beta, grade_scale):
    global LAST_RESULTS
    import ml_dtypes
    from concourse.bass_utils import run_bass_kernel_spmd

    bf16 = ml_dtypes.bfloat16

    if "nc" not in _CACHE:
        _CACHE["nc"] = _build_program()
    nc = _CACHE["nc"]

    x = np.asarray(x)
    assert x.shape == (B, NB, MV) and x.dtype == np.float32, (x.shape, x.dtype)
    gamma = np.asarray(gamma, dtype=np.float32)
    beta = np.asarray(beta, dtype=np.float32)
    grade_scale = np.asarray(grade_scale, dtype=np.float32)

    gg = grade_scale[:, None] * gamma          # [9, 64]
    gb = grade_scale[:, None] * beta           # [9, 64]

    # mv-major per nb-block: [64, 256, 4096] bf16
    x_t = np.ascontiguousarray(x.transpose(1, 2, 0)).astype(bf16)
    in_maps = []
    for i in range(N_CORES):
        sl = slice(i * NB_PER_CORE, (i + 1) * NB_PER_CORE)
        in_maps.append({
            "x": np.ascontiguousarray(x_t[sl]),
            "gg": np.ascontiguousarray(gg[:, sl]),
            "gb": np.ascontiguousarray(gb[:, sl]),
        })

    want_trace = bool(int(os.environ.get("KERNEL_TRACE", "0") or "0"))
    if want_trace:
        # tracing under axon needs the NTFF hook; fall back cleanly if absent
        try:
            from antenv.axon_hooks import get_axon_ntff_profile_hook
            want_trace = get_axon_ntff_profile_hook() is not None
        except Exception:
            want_trace = False
    # retry: the axon relay occasionally returns a transient
    # NRT_EXEC_UNIT_UNRECOVERABLE; a rerun succeeds
    last_exc = None
    for _attempt in range(3):
        try:
            res = run_bass_kernel_spmd(
                nc, in_maps, core_ids=list(range(N_CORES)), trace=want_trace,
            )
            break
        except Exception as e:
            last_exc = e
            import time as _time
            _time.sleep(2.0)
    else:
        raise last_exc
    LAST_RESULTS = res

    out_t = np.concatenate(
        [np.asarray(res.results[i]["out"]) for i in range(N_CORES)], axis=0
    )                                          # [64, 256, 4096] bf16
    out = np.ascontiguousarray(
        out_t.transpose(2, 0, 1)
    ).astype(np.float32)                       # [4096, 64, 256]
    return out
